# revision 3
# baseline (speedup 1.0000x reference)
"""CrossViewFusion Trainium2 kernel.

Computation (per batch element, data-parallel over B=8 across 8 cores):
  x1f = mean_pool4x4(x1)            [C,1024]   (pooled sums, /16 folded into Wk,Wv)
  qT  = x2f^T @ (Wq/32)^T           [1024,C]   (1/h attn scale folded into Wq)
  kT  = x1s^T @ (Wk/16)^T           [1024,C]
  v   = (Wv/16) @ x1s               [C,1024]
  aT  = kT^T-contract-> exp(q.k)    [C1,C2]    (attn transposed; softmax denom via
  s   = ones^T @ aT                 [C2]        ones-matmul; normalization applied
  out = (aT^T @ v) * (1/s) + x2                post-GEMM as per-partition scale)

All GEMMs run in bf16 on the PE array (fp32 accumulate in PSUM); pooling,
softmax denominators, normalization and the residual stay in fp32.
"""

import sys
from contextlib import ExitStack

if "/opt/trn_rl_repo" not in sys.path:
    sys.path.insert(0, "/opt/trn_rl_repo")

import numpy as np

import concourse.bass as bass
import concourse.tile as tile
from concourse import bacc, bass_utils, masks, mybir

FP32 = mybir.dt.float32
BF16 = mybir.dt.bfloat16
AX = mybir.AxisListType
AF = mybir.ActivationFunctionType

NCORES = 8

# Problem shape (per core / batch element)
C = 768            # channels (C1 == C2)
P = 128            # partition size
CT = C // P        # channel tiles
HW = 32            # pooled spatial side
N = HW * HW        # pooled spatial size (1024)
NT = N // P        # n-chunks for lhsT free dim (8)
SRC = 128          # source spatial side of x1
POOL = 4           # pool factor
CHUNK_ROWS = 16    # source rows per stream chunk
CHUNK = CHUNK_ROWS * SRC          # elems per partition per chunk (2048)
NCHUNK = SRC // CHUNK_ROWS        # stream chunks per channel tile (8)
PH = CHUNK_ROWS // POOL           # pooled rows per chunk (4)


def _col_splits(total, bank=512):
    off = 0
    out = []
    while off < total:
        w = min(bank, total - off)
        out.append((off, w))
        off += w
    return out


def build_program(reps=1):
    nc = bacc.Bacc("TRN2", target_bir_lowering=False, debug=False)

    x1_d = nc.dram_tensor("x1", [C, SRC, SRC], FP32, kind="ExternalInput").ap()
    x2_d = nc.dram_tensor("x2", [C, N], FP32, kind="ExternalInput").ap()
    wq_d = nc.dram_tensor("wq", [C, C], FP32, kind="ExternalInput").ap()
    wk_d = nc.dram_tensor("wk", [C, C], FP32, kind="ExternalInput").ap()
    wv_d = nc.dram_tensor("wv", [C, C], FP32, kind="ExternalInput").ap()
    out_d = nc.dram_tensor("out", [C, N], FP32, kind="ExternalOutput").ap()

    with tile.TileContext(nc) as tc:
        with ExitStack() as ctx:
            ent = ctx.enter_context
            const_pool = ent(tc.tile_pool(name="const", bufs=1))
            wstage = ent(tc.tile_pool(name="wstage", bufs=CT))
            wT_pool = ent(tc.tile_pool(name="wT", bufs=3 * CT))
            x2f_pool = ent(tc.tile_pool(name="x2f", bufs=CT))
            x2b_pool = ent(tc.tile_pool(name="x2b", bufs=CT))
            stream_pool = ent(tc.tile_pool(name="stream", bufs=3))
            x1sf_pool = ent(tc.tile_pool(name="x1sf", bufs=2))
            x1sb_pool = ent(tc.tile_pool(name="x1sb", bufs=CT))
            qT_pool = ent(tc.tile_pool(name="qT", bufs=NT))
            kT_pool = ent(tc.tile_pool(name="kT", bufs=NT))
            v_pool = ent(tc.tile_pool(name="vp", bufs=CT))
            expT_pool = ent(tc.tile_pool(name="expT", bufs=CT))
            rcp_pool = ent(tc.tile_pool(name="rcp", bufs=CT))
            out_pool = ent(tc.tile_pool(name="ost", bufs=4))
            ps_wide = ent(tc.tile_pool(name="ps_wide", bufs=2, space="PSUM"))
            ps_half = ent(tc.tile_pool(name="ps_half", bufs=2, space="PSUM"))
            ps_sum = ent(tc.tile_pool(name="ps_sum", bufs=2, space="PSUM"))
            ident = const_pool.tile([P, P], FP32)
            masks.make_identity(nc, ident[:])
            ones = const_pool.tile([P, 1], BF16)
            nc.gpsimd.memset(ones[:], 1.0)

            for _ in range(reps):
                # ---- load + transpose + bf16-convert weights (scales folded) ----
                wT = {}
                for name, w_d, scale in (
                    ("q", wq_d, 1.0 / HW),
                    ("k", wk_d, 1.0 / (POOL * POOL)),
                    ("v", wv_d, 1.0 / (POOL * POOL)),
                ):
                    rows = []
                    for ot in range(CT):
                        t = wstage.tile([P, C], FP32)
                        nc.sync.dma_start(t[:], w_d[ot * P:(ot + 1) * P, :])
                        rows.append(t)
                    tiles = []
                    for ct in range(CT):
                        ps = ps_wide.tile([P, C], FP32)
                        for ot in range(CT):
                            nc.tensor.transpose(
                                ps[:, ot * P:(ot + 1) * P],
                                rows[ot][:, ct * P:(ct + 1) * P],
                                ident[:],
                            )
                        wt = wT_pool.tile([P, C], BF16)
                        nc.scalar.activation(wt[:], ps[:], AF.Copy, scale=scale)
                        tiles.append(wt)
                    wT[name] = tiles

                # ---- load x2, convert to bf16 ----
                x2f, x2b = [], []
                for ct in range(CT):
                    t = x2f_pool.tile([P, N], FP32)
                    nc.sync.dma_start(t[:], x2_d[ct * P:(ct + 1) * P, :])
                    b = x2b_pool.tile([P, N], BF16)
                    nc.scalar.activation(b[:], t[:], AF.Copy)
                    x2f.append(t)
                    x2b.append(b)

                # ---- qT[n, c2] GEMM (independent of x1) ----
                qT = []
                for nt in range(NT):
                    ps = ps_wide.tile([P, C], FP32)
                    for ct in range(CT):
                        lhsT = x2b[ct][:, nt * P:(nt + 1) * P]
                        for off, w in _col_splits(C):
                            nc.tensor.matmul(
                                ps[:, off:off + w], lhsT, wT["q"][ct][:, off:off + w],
                                start=(ct == 0), stop=(ct == CT - 1),
                            )
                    qt = qT_pool.tile([P, C], BF16)
                    nc.scalar.activation(qt[:], ps[:], AF.Copy)
                    qT.append(qt)

                # ---- stream x1, pool 4x4 (sums; /16 folded into Wk/Wv) ----
                x1sb = []
                for ct in range(CT):
                    xs = x1sf_pool.tile([P, N], FP32)
                    for j in range(NCHUNK):
                        st = stream_pool.tile([P, CHUNK], FP32)
                        nc.sync.dma_start(
                            st[:],
                            x1_d[ct * P:(ct + 1) * P,
                                 j * CHUNK_ROWS:(j + 1) * CHUNK_ROWS, :],
                        )
                        src = st[:].rearrange(
                            "p (h ph w pw) -> p h w ph pw",
                            h=PH, ph=POOL, w=HW, pw=POOL,
                        )
                        nc.vector.reduce_sum(
                            xs[:, j * PH * HW:(j + 1) * PH * HW], src, axis=AX.XY,
                        )
                    xb = x1sb_pool.tile([P, N], BF16)
                    nc.scalar.activation(xb[:], xs[:], AF.Copy)
                    x1sb.append(xb)

                # ---- kT[n, c1] GEMM ----
                kT = []
                for nt in range(NT):
                    ps = ps_wide.tile([P, C], FP32)
                    for ct in range(CT):
                        lhsT = x1sb[ct][:, nt * P:(nt + 1) * P]
                        for off, w in _col_splits(C):
                            nc.tensor.matmul(
                                ps[:, off:off + w], lhsT, wT["k"][ct][:, off:off + w],
                                start=(ct == 0), stop=(ct == CT - 1),
                            )
                    kt = kT_pool.tile([P, C], BF16)
                    nc.scalar.activation(kt[:], ps[:], AF.Copy)
                    kT.append(kt)

                # ---- v[c1, n] GEMM ----
                v = []
                for ot in range(CT):
                    vt = v_pool.tile([P, N], BF16)
                    for off, w in _col_splits(N):
                        ps = ps_half.tile([P, 512], FP32)
                        for ct in range(CT):
                            nc.tensor.matmul(
                                ps[:, :w], wT["v"][ct][:, ot * P:(ot + 1) * P],
                                x1sb[ct][:, off:off + w],
                                start=(ct == 0), stop=(ct == CT - 1),
                            )
                        nc.scalar.activation(vt[:, off:off + w], ps[:, :w], AF.Copy)
                    v.append(vt)

                # ---- attnT[c1, c2] = exp(sum_n kT qT) ----
                expT = []
                for c1t in range(CT):
                    ps = ps_wide.tile([P, C], FP32)
                    for nt in range(NT):
                        lhsT = kT[nt][:, c1t * P:(c1t + 1) * P]
                        for off, w in _col_splits(C):
                            nc.tensor.matmul(
                                ps[:, off:off + w], lhsT, qT[nt][:, off:off + w],
                                start=(nt == 0), stop=(nt == NT - 1),
                            )
                    et = expT_pool.tile([P, C], BF16)
                    nc.scalar.activation(et[:], ps[:], AF.Exp)
                    expT.append(et)

                # ---- softmax denominators: colsum over c1 via ones-matmul ----
                rcp = []
                for c2t in range(CT):
                    pss = ps_sum.tile([P, 1], FP32)
                    for c1t in range(CT):
                        nc.tensor.matmul(
                            pss[:], expT[c1t][:, c2t * P:(c2t + 1) * P], ones[:],
                            start=(c1t == 0), stop=(c1t == CT - 1),
                        )
                    r = rcp_pool.tile([P, 1], FP32)
                    nc.vector.reciprocal(r[:], pss[:])
                    rcp.append(r)

                # ---- out[c2, n] = (expT^T @ v) * rcp + x2 ----
                for c2t in range(CT):
                    for off, w in _col_splits(N):
                        ps = ps_half.tile([P, 512], FP32)
                        for c1t in range(CT):
                            nc.tensor.matmul(
                                ps[:, :w], expT[c1t][:, c2t * P:(c2t + 1) * P],
                                v[c1t][:, off:off + w],
                                start=(c1t == 0), stop=(c1t == CT - 1),
                            )
                        o = out_pool.tile([P, 512], FP32)
                        nc.scalar.activation(o[:, :w], ps[:, :w], AF.Copy,
                                             scale=rcp[c2t][:])
                        nc.vector.tensor_add(o[:, :w], o[:, :w],
                                             x2f[c2t][:, off:off + w])
                        nc.sync.dma_start(
                            out_d[c2t * P:(c2t + 1) * P, off:off + w], o[:, :w],
                        )

    nc.compile()
    return nc


_cache = {}


def _get_program(reps=1):
    if reps not in _cache:
        _cache[reps] = build_program(reps)
    return _cache[reps]


def kernel(x1, x2, Wq, Wk, Wv):
    B = x1.shape[0]
    assert B == NCORES
    nc = _get_program()
    in_maps = [
        {
            "x1": np.ascontiguousarray(x1[b]),
            "x2": np.ascontiguousarray(x2[b].reshape(C, N)),
            "wq": np.ascontiguousarray(Wq),
            "wk": np.ascontiguousarray(Wk),
            "wv": np.ascontiguousarray(Wv),
        }
        for b in range(B)
    ]
    res = bass_utils.run_bass_kernel_spmd(nc, in_maps, core_ids=list(range(NCORES)))
    out = np.stack([res.results[b]["out"].reshape(C, HW, HW) for b in range(B)])
    return out.astype(np.float32)


# revision 23
# speedup vs baseline: 196.1266x; 196.1266x over previous
"""CrossViewFusion Trainium2 kernel.

Computation (per batch element, data-parallel over B=8 across 8 cores):
  x1s = sum_pool4x4(x1)             [C,1024]   (pool /16 folded into Wk,Wv)
  qT  = x2f^T @ (Wq/32)^T           [1024,C]   (1/h attn scale folded into Wq)
  kT  = x1s^T @ (Wk/16)^T           [1024,C]
  v   = (Wv/16) @ x1s               [C,1024]
  aT  = exp(kT^T-contract-qT)       [C1,C2]    (attn transposed; softmax denom via
  s   = ones-matmul colsum          [C2]        ones-matmul; normalization applied
  out = (aT^T @ v) * (1/s) + x2                post-GEMM as per-partition scale)

All GEMMs run in bf16 on the PE array (fp32 accumulate in PSUM); pooling and
softmax denominators stay in fp32.  The k/v channel contraction is split into
phase A (channel tiles 0..3, overlapped with the x1 stream) and phase B
(tiles 4..5 + combine, after the stream).
"""

import sys
from contextlib import ExitStack

if "/opt/trn_rl_repo" not in sys.path:
    sys.path.insert(0, "/opt/trn_rl_repo")

import numpy as np

import concourse.bass as bass
import concourse.tile as tile
from concourse import bacc, bass_utils, masks, mybir

FP32 = mybir.dt.float32
BF16 = mybir.dt.bfloat16
AX = mybir.AxisListType
AF = mybir.ActivationFunctionType

NCORES = 8

# Problem shape (per core / batch element)
C = 768            # channels (C1 == C2)
P = 128            # partition size
CT = C // P        # channel tiles
HW = 32            # pooled spatial side
N = HW * HW        # pooled spatial size (1024)
NT = N // P        # n-chunks for lhsT free dim (8)
SRC = 128          # source spatial side of x1
POOL = 4           # pool factor
CHUNK_ROWS = 16    # source rows per stream chunk
CHUNK = CHUNK_ROWS * SRC          # elems per partition per chunk (2048)
NCHUNK = SRC // CHUNK_ROWS        # stream chunks per channel tile (8)
PH = CHUNK_ROWS // POOL           # pooled rows per chunk (4)
PHASE_A = 4        # channel tiles contracted during the stream (k/v phase A)


def _col_splits(total, bank=512):
    off = 0
    out = []
    while off < total:
        w = min(bank, total - off)
        out.append((off, w))
        off += w
    return out


def build_program(reps=1, loop_reps=None, timing_mode=False):
    """reps: python-unrolled repetitions. loop_reps: on-device For_i repetitions
    (for timing; same program size regardless of trip count). timing_mode makes
    the inputs Internal DRAM (uninitialized, nothing shipped per call)."""
    nc = bacc.Bacc("TRN2", target_bir_lowering=False, debug=False)

    kind = "Internal" if timing_mode else "ExternalInput"
    x1_d = nc.dram_tensor("x1", [C, SRC, SRC], FP32, kind=kind).ap()
    x2_d = nc.dram_tensor("x2", [C, N], FP32, kind=kind).ap()
    wq_d = nc.dram_tensor("wq", [C, C], FP32, kind=kind).ap()
    wk_d = nc.dram_tensor("wk", [C, C], FP32, kind=kind).ap()
    wv_d = nc.dram_tensor("wv", [C, C], FP32, kind=kind).ap()
    out_d = nc.dram_tensor("out", [C, N], FP32, kind="ExternalOutput").ap()

    with tile.TileContext(nc) as tc:
        with ExitStack() as ctx:
            ent = ctx.enter_context
            const_pool = ent(tc.tile_pool(name="const", bufs=1))
            wstage = ent(tc.tile_pool(name="wstage", bufs=1))
            wT_pool = ent(tc.tile_pool(name="wT", bufs=3 * CT))
            x2f_pool = ent(tc.tile_pool(name="x2f", bufs=1))
            x2b_pool = ent(tc.tile_pool(name="x2b", bufs=CT))
            stream_pool = ent(tc.tile_pool(name="stream", bufs=3))
            x1sb_pool = ent(tc.tile_pool(name="x1sb", bufs=CT))
            qT_pool = ent(tc.tile_pool(name="qT", bufs=NT))
            kT_pool = ent(tc.tile_pool(name="kT", bufs=NT))
            v_pool = ent(tc.tile_pool(name="vp", bufs=CT))
            expT_pool = ent(tc.tile_pool(name="expT", bufs=CT))
            rcp_pool = ent(tc.tile_pool(name="rcp", bufs=CT))
            out_pool = ent(tc.tile_pool(name="ost", bufs=4))
            ps_wide = ent(tc.tile_pool(name="ps_wide", bufs=2, space="PSUM"))
            ps_half = ent(tc.tile_pool(name="ps_half", bufs=2, space="PSUM"))
            ps_sum = ent(tc.tile_pool(name="ps_sum", bufs=2, space="PSUM"))

            ident = const_pool.tile([P, P], FP32)
            masks.make_identity(nc, ident[:])
            ones = const_pool.tile([P, 1], BF16)
            nc.gpsimd.memset(ones[:], 1.0)

            def load_wT(w_d, scale):
                """Load W [C,C] f32 with ONE row-folded SWDGE DMA (partition
                p holds rows p, p+128, ..), then PE-transpose + bf16-convert.
                Returns transposed tiles [c partition, o free], scaled."""
                t = wstage.tile([P, CT * C], FP32)
                src = w_d.rearrange("(b p) c -> p b c", p=P)
                nc.scalar.dma_start(t[:], src)
                tiles = []
                for ct in range(CT):
                    ps = ps_wide.tile([P, C], FP32)
                    for ot in range(CT):
                        nc.tensor.transpose(
                            ps[:, ot * P:(ot + 1) * P],
                            t[:, ot * C + ct * P:ot * C + (ct + 1) * P],
                            ident[:],
                        )
                    wt = wT_pool.tile([P, C], BF16)
                    nc.scalar.activation(wt[:], ps[:], AF.Copy, scale=scale)
                    tiles.append(wt)
                return tiles

            def stream_ct(ct):
                """Stream + pool one x1 channel tile into bf16 sums."""
                xb = x1sb_pool.tile([P, N], BF16)
                for j in range(NCHUNK):
                    st = stream_pool.tile([P, CHUNK], FP32)
                    nc.sync.dma_start(
                        st[:],
                        x1_d[ct * P:(ct + 1) * P,
                             j * CHUNK_ROWS:(j + 1) * CHUNK_ROWS, :],
                    )
                    src = st[:].rearrange(
                        "p (h ph w pw) -> p h w ph pw",
                        h=PH, ph=POOL, w=HW, pw=POOL,
                    )
                    with nc.allow_low_precision(
                        reason="pooled sums rounded to bf16 for the GEMMs"
                    ):
                        nc.vector.reduce_sum(
                            xb[:, j * PH * HW:(j + 1) * PH * HW], src, axis=AX.XY,
                        )
                return xb

            def kT_phase(wTk, x1sb, cts, kT, first):
                """k-GEMM over channel tiles `cts`.  first: ACT-convert psum
                into kT tiles; else DVE-add psum onto the phase-A partials."""
                for nt in range(NT):
                    ps = ps_wide.tile([P, C], FP32)
                    for i, ct in enumerate(cts):
                        lhsT = x1sb[ct][:, nt * P:(nt + 1) * P]
                        for off, w in _col_splits(C):
                            nc.tensor.matmul(
                                ps[:, off:off + w], lhsT, wTk[ct][:, off:off + w],
                                start=(i == 0), stop=(i == len(cts) - 1),
                            )
                    if first:
                        kt = kT_pool.tile([P, C], BF16)
                        nc.scalar.activation(kt[:], ps[:], AF.Copy)
                        kT.append(kt)
                    else:
                        nc.vector.tensor_add(kT[nt][:], kT[nt][:], ps[:])

            def v_phase(wTv, x1sb, cts, v, first):
                for ot in range(CT):
                    if first:
                        vt = v_pool.tile([P, N], BF16)
                        v.append(vt)
                    for off, w in _col_splits(N):
                        ps = ps_half.tile([P, 512], FP32)
                        for i, ct in enumerate(cts):
                            nc.tensor.matmul(
                                ps[:, :w], wTv[ct][:, ot * P:(ot + 1) * P],
                                x1sb[ct][:, off:off + w],
                                start=(i == 0), stop=(i == len(cts) - 1),
                            )
                        if first:
                            nc.scalar.activation(
                                v[ot][:, off:off + w], ps[:, :w], AF.Copy)
                        else:
                            nc.vector.tensor_add(
                                v[ot][:, off:off + w], v[ot][:, off:off + w],
                                ps[:, :w])

            def body():
                # Weights + x2 load on the SWDGE queue (independent of the
                # x1 stream on the HWDGE queue).
                wTk = load_wT(wk_d, 1.0 / (POOL * POOL))
                wTv = load_wT(wv_d, 1.0 / (POOL * POOL))
                wTq = load_wT(wq_d, 1.0 / HW)
                x2fold = x2f_pool.tile([P, CT * N], FP32)
                nc.scalar.dma_start(
                    x2fold[:], x2_d.rearrange("(b p) n -> p b n", p=P))
                x2f, x2b = [], []
                for ct in range(CT):
                    t = x2fold[:, ct * N:(ct + 1) * N]
                    b = x2b_pool.tile([P, N], BF16)
                    nc.scalar.activation(b[:], t[:], AF.Copy)
                    x2f.append(t)
                    x2b.append(b)

                # Stream phase-A channel tiles.
                x1sb = [stream_ct(ct) for ct in range(PHASE_A)]

                # k/v phase A (contracts ct 0..PHASE_A-1) — overlaps the
                # remaining stream.
                kT, v = [], []
                kT_phase(wTk, x1sb, range(PHASE_A), kT, first=True)
                v_phase(wTv, x1sb, range(PHASE_A), v, first=True)

                qT = []
                for nt in range(NT):
                    ps = ps_wide.tile([P, C], FP32)
                    for ct in range(CT):
                        lhsT = x2b[ct][:, nt * P:(nt + 1) * P]
                        for off, w in _col_splits(C):
                            nc.tensor.matmul(
                                ps[:, off:off + w], lhsT, wTq[ct][:, off:off + w],
                                start=(ct == 0), stop=(ct == CT - 1),
                            )
                    qt = qT_pool.tile([P, C], BF16)
                    nc.scalar.activation(qt[:], ps[:], AF.Copy)
                    qT.append(qt)

                # Stream the remaining channel tiles; k/v phase B combines.
                for ct in range(PHASE_A, CT):
                    x1sb.append(stream_ct(ct))
                kT_phase(wTk, x1sb, range(PHASE_A, CT), kT, first=False)

                # attnT[c1, c2] = exp(sum_n kT qT)
                expT = []
                for c1t in range(CT):
                    ps = ps_wide.tile([P, C], FP32)
                    for nt in range(NT):
                        lhsT = kT[nt][:, c1t * P:(c1t + 1) * P]
                        for off, w in _col_splits(C):
                            nc.tensor.matmul(
                                ps[:, off:off + w], lhsT, qT[nt][:, off:off + w],
                                start=(nt == 0), stop=(nt == NT - 1),
                            )
                    et = expT_pool.tile([P, C], BF16)
                    nc.scalar.activation(et[:], ps[:], AF.Exp)
                    expT.append(et)

                # v phase B (needed only by the out-GEMM, after exp)
                v_phase(wTv, x1sb, range(PHASE_A, CT), v, first=False)

                # softmax denominators: colsum over c1 via ones-matmul
                rcp = []
                for c2t in range(CT):
                    pss = ps_sum.tile([P, 1], FP32)
                    for c1t in range(CT):
                        nc.tensor.matmul(
                            pss[:], expT[c1t][:, c2t * P:(c2t + 1) * P], ones[:],
                            start=(c1t == 0), stop=(c1t == CT - 1),
                        )
                    r = rcp_pool.tile([P, 1], FP32)
                    nc.vector.reciprocal(r[:], pss[:])
                    rcp.append(r)

                # out[c2, n] = (expT^T @ v) * rcp + x2
                for c2t in range(CT):
                    for off, w in _col_splits(N):
                        ps = ps_half.tile([P, 512], FP32)
                        for c1t in range(CT):
                            nc.tensor.matmul(
                                ps[:, :w], expT[c1t][:, c2t * P:(c2t + 1) * P],
                                v[c1t][:, off:off + w],
                                start=(c1t == 0), stop=(c1t == CT - 1),
                            )
                        o = out_pool.tile([P, 512], FP32)
                        nc.vector.scalar_tensor_tensor(
                            o[:, :w], ps[:, :w], rcp[c2t][:],
                            x2f[c2t][:, off:off + w],
                            op0=mybir.AluOpType.mult, op1=mybir.AluOpType.add)
                        nc.sync.dma_start(
                            out_d[c2t * P:(c2t + 1) * P, off:off + w], o[:, :w],
                        )

            if loop_reps is not None:
                with tc.For_i(0, loop_reps, 1,
                              hint_engines=(mybir.EngineType.PE,)):
                    body()
            else:
                for _ in range(reps):
                    body()

    nc.compile()
    return nc


_cache = {}


def _get_program(reps=1):
    if reps not in _cache:
        _cache[reps] = build_program(reps)
    return _cache[reps]


def kernel(x1, x2, Wq, Wk, Wv):
    B = x1.shape[0]
    assert B == NCORES
    nc = _get_program()
    in_maps = [
        {
            "x1": np.ascontiguousarray(x1[b]),
            "x2": np.ascontiguousarray(x2[b].reshape(C, N)),
            "wq": np.ascontiguousarray(Wq),
            "wk": np.ascontiguousarray(Wk),
            "wv": np.ascontiguousarray(Wv),
        }
        for b in range(B)
    ]
    res = bass_utils.run_bass_kernel_spmd(nc, in_maps, core_ids=list(range(NCORES)))
    out = np.stack([res.results[b]["out"].reshape(C, HW, HW) for b in range(B)])
    return out.astype(np.float32)
